# revision 1
# baseline (speedup 1.0000x reference)
"""ChannelAttention (Softmax2d-over-batch) Trainium2 kernel, 8-core SPMD.

v4: fully SBUF-resident bf16 GEMM path + consolidated DMA.
Data-parallel over batch (4 samples/core); the only cross-core coupling
is Z[c,d] = sum_b exp(scores[b,c,d] - SHIFT), reduced with a bf16
AllReduce split into two chunks so it hides under compute.

Design notes (from tile-sim traces + HW differentials):
- At N>=256 free-dim, bf16 matmuls stream 1 elem/cycle (same as fp32r),
  so bf16 changes no PE time but halves SBUF/DMA cost -> E (102 KB/p),
  K/Q (40), V (20) all stay resident.
- DMA is issued as ~38 large transfers per pass (weights/x/out as whole
  [P, KC, *] tensors) instead of ~170 small tiles: transfers >=1 MB run
  at ~78% of the 358 GB/s per-core HBM limit vs ~40% for 128 KB tiles,
  and each dma_start costs ~0.5-2 us of issuing-engine queue time.
- Phase B is ACT-bound (exp ~66 us vs 43 us of PE scores work): half of
  the V GEMMs are interleaved into B to keep PE fed while ACT drains
  PSUM; the other half runs after B to hide the second AllReduce chunk.
- S is kept bf16 end-to-end (pairwise-tree sum on DVE, bf16 AllReduce);
  Z is cast bf16->f32 during the s_out load via a GpSimd casting DMA,
  so the reciprocal needs no separate conversion op.
- Engine balance: K/Q/att PSUM->SBUF copies on ACT, V copies on DVE via
  tensor_scalar_add (folds bv, removing 40 rank-1 matmuls), refine bias
  folded into affine_then_add (removing 40 more).

Per core:
  A:   Kt[b], Qt[b] = ((W @ x_b)^T + b) -> SBUF bf16 [HW, C]
  B:   (dt-outer) scoresT[d,c] -> exp -> E bf16; S[dt] = sum_b E_b[dt]
       -> DRAM bf16; V GEMMs for vct=dt interleaved for dt<5
  AR0: AllReduce(S[0:5]) issued mid-B; AR1: AllReduce(S[5:10]) after B
  C1b: V GEMMs vct 5..9 (hide AR1)
  R:   R = 1/Z (DVE accurate recip), stored bf16
  C2:  E *= R in place (DVE+GpSimd, pipelined one sample ahead);
       att[b] = attnT-contract @ V[b] -> SBUF bf16
  C3:  out[b] = alpha * (Wr @ att[b]) + (alpha*br + x_b), buffered in
       SBUF and stored with 4 large DMAs
"""

import os

import numpy as np
import ml_dtypes

import concourse.bass as bass
import concourse.tile as tile
from concourse import bacc, mybir
from concourse import bass_utils

B, C, S, HW = 32, 1280, 16, 256
P = 128
KC = C // P          # 10 chunks of the channel dim
NCORES = 8
BL = B // NCORES     # 4 samples per core
SHIFT = 45.0
CGROUPS = [(0, 512), (512, 512), (1024, 256)]  # psum-bank-sized column groups
F32 = mybir.dt.float32
BF16 = mybir.dt.bfloat16
AF = mybir.ActivationFunctionType

_CACHE = {}
AR_MODE = os.environ.get("KERNEL_AR_MODE", "split")


def _emit(nc, tc, io, alpha, dbg):
    """Emit one full forward pass (phases A..C3)."""
    ones, bvc, brc = io["ones_t"], io["bvc_t"], io["brc_t"]
    xb_d, x_d = io["xb_d"], io["x_d"]
    wk_d, wq_d, wv_d, wr_d = io["wk_d"], io["wq_d"], io["wv_d"], io["wr_d"]
    s_in, s_out, out_d = io["s_in"], io["s_out"], io["out_d"]

    # Pool plan (SBUF = two stacks; each side closes in reverse open order):
    #   left:  xpool(xb 20K, A..C1b) > wV(12.8K) > ktqt(40K, A..B) > wA(56K)
    #          then sstage(B), zbuf(R), then attp(20K, C2..C3) > wrp/xop
    #   right: epool(E 102.4K, B..C2) > vpool(V 20K, B..C2) > rpool(25.6K)
    xpool_ctx = tc.tile_pool(name="xpool", bufs=1, side="left")
    xpool = xpool_ctx.__enter__()                    # 20 KB/p, A..C1b
    xb_sb = xpool.tile([P, BL, KC, HW], BF16, tag="xb")
    nc.sync.dma_start(
        xb_sb[:], xb_d.ap().rearrange("b (k p) n -> p b k n", p=P)
    )

    # wV half-tensor pool lives B..C1b; under ktqt on the left stack
    # (ktqt closes first). One slot: half2's load waits for half1's
    # release after the fused-B V GEMMs.
    wv_ctx = tc.tile_pool(name="wV", bufs=1, side="left")
    wpV = wv_ctx.__enter__()                         # 12.8 KB/p

    # ========= phase A: Kt, Qt resident in SBUF (bf16) =========
    ktqt_ctx = tc.tile_pool(name="ktqt", bufs=1, side="left")
    ktqtp = ktqt_ctx.__enter__()
    kt_sb = ktqtp.tile([P, 2, BL, C], BF16, tag="kt")   # 20 KB/p
    qt_sb = ktqtp.tile([P, 2, BL, C], BF16, tag="qt")   # 20 KB/p
    with (
        tc.tile_pool(name="wA", bufs=1, side="left") as wpA,   # 56.2 KB/p
        tc.tile_pool(name="psumA", bufs=2, space="PSUM") as psA,
    ):
        wk_sb = wpA.tile([P, KC, C], BF16, tag="wk")
        nc.sync.dma_start(wk_sb[:], wk_d.ap().rearrange("(k p) n -> p k n", p=P))
        wq_sb = wpA.tile([P, KC, C], BF16, tag="wq")
        nc.sync.dma_start(wq_sb[:], wq_d.ap().rearrange("(k p) n -> p k n", p=P))
        brow = {}
        for nm in ("bk", "bq"):
            t = wpA.tile([1, C], BF16, tag=f"row_{nm}", name=f"brow_{nm}")
            nc.sync.dma_start(t[:], io[nm].ap())
            brow[nm] = t
        for w_sb, bias, dest in ((wk_sb, "bk", kt_sb), (wq_sb, "bq", qt_sb)):
            for cgs, cgl in CGROUPS:
                for b in range(BL):
                    for hwt in range(2):
                        ps = psA.tile([P, 512], F32, tag="psA")
                        for k in range(KC):
                            nc.tensor.matmul(
                                ps[:, :cgl],
                                xb_sb[:, b, k, hwt * P:(hwt + 1) * P],
                                w_sb[:, k, cgs:cgs + cgl],
                                start=(k == 0),
                                stop=False,
                            )
                        nc.tensor.matmul(
                            ps[:, :cgl],
                            ones[:, :P],
                            brow[bias][:, cgs:cgs + cgl],
                            start=False,
                            stop=True,
                        )
                        nc.scalar.copy(
                            dest[:, hwt, b, cgs:cgs + cgl], ps[:, :cgl]
                        )
                        if dbg and b == 0 and hwt == 0 and dest is kt_sb:
                            nc.sync.dma_start(
                                io["dbg_kt"].ap()[:, cgs:cgs + cgl],
                                dest[:, hwt, b, cgs:cgs + cgl],
                            )

    # ========= phase B (+ first half of C1 interleaved) =========
    epool_ctx = tc.tile_pool(name="epool", bufs=1, side="right")
    epool = epool_ctx.__enter__()                    # 102.4 KB/p, B..C2
    e_sb = epool.tile([P, BL, KC, C], BF16, tag="E")
    vpool_ctx = tc.tile_pool(name="vpool", bufs=1, side="right")
    vpool = vpool_ctx.__enter__()                    # 20 KB/p, B..C2
    v_sb = vpool.tile([P, BL, KC, HW], BF16, tag="V")

    def v_half_load(half, name):
        # [P(cin), KC, 640(cout)] -- half of Wv^T, cols half*640..+640
        wvt = wpV.tile([P, KC, 5 * P], BF16, tag="wV", name=name)
        nc.sync.dma_start(
            wvt[:],
            wv_d.ap()[:, half * 5 * P:(half + 1) * 5 * P].rearrange(
                "(k p) n -> p k n", p=P),
        )
        return wvt

    def v_gemms(vct, wvt, psV):
        col = (vct % 5) * P
        for b in range(BL):
            ps = psV.tile([P, HW], F32, tag="psV")
            for ci in range(KC):
                nc.tensor.matmul(
                    ps[:], wvt[:, ci, col:col + P], xb_sb[:, b, ci],
                    start=(ci == 0), stop=(ci == KC - 1),
                )
            # V = psum + bv  (bias folded into the drain copy)
            nc.vector.tensor_scalar_add(
                v_sb[:, b, vct], ps[:], bvc[:, vct:vct + 1]
            )
            if dbg and b == 0 and vct == 0:
                nc.sync.dma_start(io["dbg_v"].ap(), v_sb[:, b, vct])

    with (
        tc.tile_pool(name="sstage", bufs=2, side="left") as sstp,  # 7.5 KB/p
        tc.tile_pool(name="psumB", bufs=3, space="PSUM") as psB,
        tc.tile_pool(name="psumV", bufs=2, space="PSUM") as psV,
    ):
        wvt0 = v_half_load(0, "wvt0")
        for dt_ in range(KC):
            for b in range(BL):
                for cgs, cgl in CGROUPS:
                    ps = psB.tile([P, 512], F32, tag="psB")
                    for hwt in range(2):
                        nc.tensor.matmul(
                            ps[:, :cgl],
                            qt_sb[:, hwt, b, dt_ * P:(dt_ + 1) * P],
                            kt_sb[:, hwt, b, cgs:cgs + cgl],
                            start=(hwt == 0),
                            stop=(hwt == 1),
                        )
                    et = e_sb[:, b, dt_, cgs:cgs + cgl]
                    nc.scalar.activation(
                        et, ps[:, :cgl], AF.Exp, bias=-SHIFT, scale=1.0,
                    )
                    if dbg and b == 0 and dt_ == 0:
                        nc.sync.dma_start(
                            io["dbg_e"].ap()[:, cgs:cgs + cgl], et
                        )
            # S[dt] = (E0+E1) + (E2+E3), bf16 pairwise tree on DVE
            s01 = sstp.tile([P, C], BF16, tag="spair", name="s01")
            s23 = sstp.tile([P, C], BF16, tag="spair", name="s23")
            st = sstp.tile([P, C], BF16, tag="st", bufs=1, name="st")
            nc.vector.tensor_add(s01[:], e_sb[:, 0, dt_], e_sb[:, 1, dt_])
            nc.vector.tensor_add(s23[:], e_sb[:, 2, dt_], e_sb[:, 3, dt_])
            nc.vector.tensor_add(st[:], s01[:], s23[:])
            nc.sync.dma_start(s_in.ap()[dt_], st[:])
            if dbg and dt_ == 0:
                nc.sync.dma_start(io["dbg_s"].ap(), st[:])
            # interleave first half of the V GEMMs; keeps PE fed while
            # ACT drains exp, and leaves vct 5..9 to hide AR1
            if dt_ < 5:
                v_gemms(dt_, wvt0, psV)
            if dt_ == 4 and AR_MODE == "split":
                nc.gpsimd.collective_compute(
                    "AllReduce",
                    mybir.AluOpType.add,
                    replica_groups=[list(range(NCORES))],
                    ins=[s_in.ap()[0:5]],
                    outs=[s_out.ap()[0:5]],
                )
        if AR_MODE == "split":
            nc.gpsimd.collective_compute(
                "AllReduce",
                mybir.AluOpType.add,
                replica_groups=[list(range(NCORES))],
                ins=[s_in.ap()[5:KC]],
                outs=[s_out.ap()[5:KC]],
            )
        elif AR_MODE == "single":
            nc.gpsimd.collective_compute(
                "AllReduce",
                mybir.AluOpType.add,
                replica_groups=[list(range(NCORES))],
                ins=[s_in.ap()],
                outs=[s_out.ap()],
            )
        elif AR_MODE == "none":
            # diagnostic only: skip the collective, copy local S -> s_out
            for d2 in range(KC):
                nc.sync.dma_start(s_out.ap()[d2], s_in.ap()[d2])
    ktqt_ctx.__exit__(None, None, None)

    # ========= phase C1b: V GEMMs vct 5..9 (hide AR1) + phase R =========
    rpool_ctx = tc.tile_pool(name="rpool", bufs=1, side="right")
    rpool = rpool_ctx.__enter__()                    # 25.6 KB/p, R..C2
    r_sb = rpool.tile([P, KC, C], BF16, tag="R")

    def r_chunk(dt_, zbufp, psR):
        zt = zbufp.tile([P, C], F32, tag="Z", name="zt")
        # bf16 -> f32 cast during the DMA (gpsimd-initiated casts)
        nc.gpsimd.dma_start(zt[:], s_out.ap()[dt_])
        if dbg and dt_ == 0:
            nc.sync.dma_start(io["dbg_z"].ap(), zt[:])
        for cgs, cgl in CGROUPS:
            r32 = psR.tile([P, 512], F32, tag="R32", name="r32")
            sc_t = psR.tile([P, 512], F32, tag="Rscr", name="rscr")
            nc.vector.reciprocal_approx_accurate(
                r32[:, :cgl], zt[:, cgs:cgs + cgl], sc_t[:, :cgl]
            )
            nc.vector.tensor_copy(r_sb[:, dt_, cgs:cgs + cgl], r32[:, :cgl])
        if dbg and dt_ == 0:
            nc.sync.dma_start(io["dbg_r"].ap(), r_sb[:, dt_])

    with (
        tc.tile_pool(name="zbuf", bufs=2, side="left") as zbufp,
        tc.tile_pool(name="psumR", bufs=2, space="PSUM") as psR,
        tc.tile_pool(name="psumV2", bufs=2, space="PSUM") as psV2,
    ):
        wvt1 = v_half_load(1, "wvt1")
        for dt_ in range(5):
            r_chunk(dt_, zbufp, psR)
        for vct in range(5, KC):
            v_gemms(vct, wvt1, psV2)
        for dt_ in range(5, KC):
            r_chunk(dt_, zbufp, psR)
    wv_ctx.__exit__(None, None, None)
    xpool_ctx.__exit__(None, None, None)

    # ========= phase C2: E *= R; att[b] -> SBUF bf16 =========
    attp_ctx = tc.tile_pool(name="attp", bufs=1, side="left")
    attp = attp_ctx.__enter__()                      # 20 KB/p, C2..C3
    att_sb = attp.tile([P, BL, KC, HW], BF16, tag="att")

    def attn_muls(b):
        for dt_ in range(KC):
            eng = nc.gpsimd if dt_ % 2 == 1 else nc.vector
            eng.tensor_mul(e_sb[:, b, dt_], e_sb[:, b, dt_], r_sb[:, dt_])

    def att_gemms(b, psC2):
        for ct in range(KC):
            ps = psC2.tile([P, HW], F32, tag="psC2")
            for dt_ in range(KC):
                nc.tensor.matmul(
                    ps[:], e_sb[:, b, dt_, ct * P:(ct + 1) * P],
                    v_sb[:, b, dt_],
                    start=(dt_ == 0), stop=(dt_ == KC - 1),
                )
            nc.scalar.copy(att_sb[:, b, ct], ps[:])
            if dbg and b == 0 and ct == 0:
                nc.sync.dma_start(io["dbg_att"].ap(), att_sb[:, b, ct])

    with tc.tile_pool(name="psumC2", bufs=3, space="PSUM") as psC2:
        for b in range(BL):
            attn_muls(b)
            if b >= 1:
                att_gemms(b - 1, psC2)
        att_gemms(BL - 1, psC2)
    rpool_ctx.__exit__(None, None, None)
    vpool_ctx.__exit__(None, None, None)
    epool_ctx.__exit__(None, None, None)

    # ========= phase C3: refine + residual =========
    with (
        tc.tile_pool(name="wrp", bufs=1, side="left") as wrp,     # 25.6 KB/p
        tc.tile_pool(name="xop", bufs=1, side="left") as xop,     # 80 KB/p
        tc.tile_pool(name="psumC3", bufs=3, space="PSUM") as psC3,
    ):
        wr_sb = wrp.tile([P, KC, C], BF16, tag="wrt")
        nc.sync.dma_start(wr_sb[:], wr_d.ap().rearrange("(k p) n -> p k n", p=P))
        x_sb = xop.tile([P, BL, KC, HW], F32, tag="x32")
        nc.sync.dma_start(
            x_sb[:], x_d.ap().rearrange("b (k p) n -> p b k n", p=P)
        )
        o_sb = xop.tile([P, BL, KC, HW], F32, tag="o32")
        for ot in range(KC):
            for b in range(BL):
                ps = psC3.tile([P, HW], F32, tag="psC3")
                for ct in range(KC):
                    nc.tensor.matmul(
                        ps[:], wr_sb[:, ct, ot * P:(ot + 1) * P],
                        att_sb[:, b, ct],
                        start=(ct == 0), stop=(ct == KC - 1),
                    )
                # out = alpha * psum + (alpha*br + x)
                nc.vector.affine_then_add(
                    o_sb[:, b, ot], ps[:], x_sb[:, b, ot],
                    scale=alpha, bias=brc[:, ot:ot + 1]
                )
        nc.sync.dma_start(
            out_d.ap().rearrange("b (k p) n -> p b k n", p=P), o_sb[:]
        )
    attp_ctx.__exit__(None, None, None)


def build(alpha: float, dbg: bool = False, nrep: int = 1):
    nc = bacc.Bacc(
        "TRN2",
        target_bir_lowering=False,
        debug=False,
        enable_asserts=False,
        num_devices=NCORES,
    )

    io = {}
    io["xb_d"] = nc.dram_tensor("xb", [BL, C, HW], BF16, kind="ExternalInput")
    io["x_d"] = nc.dram_tensor("x", [BL, C, HW], F32, kind="ExternalInput")
    io["wk_d"] = nc.dram_tensor("wkt", [C, C], BF16, kind="ExternalInput")  # Wk.T
    io["wq_d"] = nc.dram_tensor("wqt", [C, C], BF16, kind="ExternalInput")
    io["wv_d"] = nc.dram_tensor("wvt", [C, C], BF16, kind="ExternalInput")
    io["wr_d"] = nc.dram_tensor("wrt", [C, C], BF16, kind="ExternalInput")
    for nm in ("bk", "bq"):
        io[nm] = nc.dram_tensor(nm, [1, C], BF16, kind="ExternalInput")
    io["bvc"] = nc.dram_tensor("bvc", [P, KC], F32, kind="ExternalInput")
    io["brc"] = nc.dram_tensor("brc", [P, KC], F32, kind="ExternalInput")
    io["ones_d"] = nc.dram_tensor("ones", [1, HW], BF16, kind="ExternalInput")
    io["out_d"] = nc.dram_tensor("out", [BL, C, HW], F32, kind="ExternalOutput")
    if dbg:
        io["dbg_kt"] = nc.dram_tensor("dbg_kt", [P, C], BF16, kind="ExternalOutput")
        io["dbg_e"] = nc.dram_tensor("dbg_e", [P, C], BF16, kind="ExternalOutput")
        io["dbg_s"] = nc.dram_tensor("dbg_s", [P, C], BF16, kind="ExternalOutput")
        io["dbg_z"] = nc.dram_tensor("dbg_z", [P, C], F32, kind="ExternalOutput")
        io["dbg_r"] = nc.dram_tensor("dbg_r", [P, C], F32, kind="ExternalOutput")
        io["dbg_v"] = nc.dram_tensor("dbg_v", [P, HW], BF16, kind="ExternalOutput")
        io["dbg_att"] = nc.dram_tensor("dbg_att", [P, HW], BF16, kind="ExternalOutput")

    io["s_in"] = nc.dram_tensor("s_in", [KC, P, C], BF16)
    io["s_out"] = nc.dram_tensor("s_out", [KC, P, C], BF16, addr_space="Shared")

    # const AP so ACT Exp can take bias=-SHIFT
    cshift = nc.alloc_sbuf_tensor("const-shift", [128, 1], F32)
    nc.gpsimd.memset(cshift.ap(), -SHIFT)
    nc.const_aps.aps[(F32, -SHIFT)] = cshift.ap()
    nc.all_engine_barrier()

    with tile.TileContext(nc) as tc:
        with tc.tile_pool(name="cpool", bufs=1, side="left") as cpool:
            # constants (live whole kernel, ~0.6 KB/p)
            ones = cpool.tile([1, HW], BF16, tag="ones")
            nc.sync.dma_start(ones[:], io["ones_d"].ap())
            bvc = cpool.tile([P, KC], F32, tag="bvc")
            nc.sync.dma_start(bvc[:], io["bvc"].ap())
            brc = cpool.tile([P, KC], F32, tag="brc")
            nc.sync.dma_start(brc[:], io["brc"].ap())
            io["ones_t"] = ones
            io["bvc_t"] = bvc
            io["brc_t"] = brc

            for _ in range(nrep):
                _emit(nc, tc, io, alpha, dbg)

    nc.compile()
    return nc


def make_in_maps(x, Wq, bq, Wk, bk, Wv, bv, Wr, br, alpha=0.1):
    bf = ml_dtypes.bfloat16
    alpha_f = float(np.asarray(alpha).reshape(-1)[0])
    xs = np.ascontiguousarray(np.asarray(x, dtype=np.float32).reshape(B, C, HW))
    xsb = xs.astype(bf)
    w = {
        "wkt": np.ascontiguousarray(np.asarray(Wk, dtype=np.float32).T.astype(bf)),
        "wqt": np.ascontiguousarray(np.asarray(Wq, dtype=np.float32).T.astype(bf)),
        "wvt": np.ascontiguousarray(np.asarray(Wv, dtype=np.float32).T.astype(bf)),
        "wrt": np.ascontiguousarray(np.asarray(Wr, dtype=np.float32).T.astype(bf)),
    }
    rows = {
        "bk": np.asarray(bk, dtype=np.float32).reshape(1, C).astype(bf),
        "bq": np.asarray(bq, dtype=np.float32).reshape(1, C).astype(bf),
    }
    # per-partition bias columns: [P, KC]; chunk ct holds channels
    # ct*P..(ct+1)*P-1 in partition order
    bvc = np.ascontiguousarray(
        np.asarray(bv, dtype=np.float32).reshape(KC, P).T)
    brc = np.ascontiguousarray(
        (alpha_f * np.asarray(br, dtype=np.float32)).reshape(KC, P).T)
    in_maps = []
    for c in range(NCORES):
        in_maps.append({
            "x": np.ascontiguousarray(xs[c * BL:(c + 1) * BL]),
            "xb": np.ascontiguousarray(xsb[c * BL:(c + 1) * BL]),
            **w,
            "ones": np.ones((1, HW), dtype=bf),
            "bvc": bvc,
            "brc": brc,
            **rows,
        })
    return in_maps


def kernel(x, Wq, bq, Wk, bk, Wv, bv, Wr, br, alpha):
    alpha_f = float(np.asarray(alpha).reshape(-1)[0])
    key = ("v4", alpha_f)
    if key not in _CACHE:
        _CACHE[key] = build(alpha_f)
    nc = _CACHE[key]

    in_maps = make_in_maps(x, Wq, bq, Wk, bk, Wv, bv, Wr, br, alpha_f)
    res = bass_utils.run_bass_kernel_spmd(nc, in_maps, core_ids=list(range(NCORES)))
    out = np.concatenate([res.results[c]["out"] for c in range(NCORES)], axis=0)
    return np.ascontiguousarray(out.reshape(B, C, S, S).astype(np.float32))



# revision 4
# speedup vs baseline: 1.0058x; 1.0058x over previous
"""ChannelAttention (Softmax2d-over-batch) Trainium2 kernel, 8-core SPMD.

v5: single fused pipeline; PE is kept continuously busy and the batch
AllReduce of S = sum_b exp(scores) is split into 3 chunks on separate
DRAM tensors, issued mid-stream so they land under the V GEMMs.

Structure (per core, 4 samples):
  kt:   Kt[b] = ((Wk @ x_b)^T + bk) -> SBUF bf16 [HW, C]    (PE 47us)
  qt0:  Qt cgroup0 (cols 0:512)                              (PE 14us)
  B:    for dt 0..9: scoresT[d,c] = Qt_dt^T Kt (both hwt)   (PE 43us)
          -> ACT exp -> E bf16; S[dt] = sum_b E_b[dt] (DVE tree)
        qt cgroups 1,2 interleaved into dt 0..5 so ACT exp
        (66us, the B bottleneck) overlaps PE qt work.
        AR chunks (gpsimd->TOPSP, Pool does nothing else):
          dt2 -> AR0(S[0:3]), dt5 -> AR1(S[3:6]), dt9 -> AR2(S[6:10])
  C1:   V GEMMs (PE 43us, N=512 over sample pairs) hide the ARs;
        per-dt: z=AR out -> f32 (ACT) -> 1/z (DVE) -> E*=R in place
        (DVE/GpSimd split), pipelined behind the AR chunk landings.
  C2:   att[b] = attnT-contract @ V[b] (PE 43us), psum->SBUF on ACT,
        att stored [P, ct, b, HW] so refine streams N=512.
  C3:   refine GEMMs (PE 43us) interleaved with C2 per sample-pair;
        out = alpha*psum + (alpha*br + x_bf16), stored per (b, ot).

SBUF plan (strict LIFO per side; ~204 KB/p peak):
  left:  cpool | xb 20K (whole kernel) | ktqt 40K (..B) | wk 25.6K (..kt)
         then sst 7.5K (B) | then wr 25.6K, o 3K, att 20K, wv 5K (C)
  right: (after kt) E 102.4K (B..att) | wq 25.6K (..dt5)
         then V 20K, zb 2.5K, r 5K (C)
"""

import os

import numpy as np
import ml_dtypes

import concourse.bass as bass
import concourse.tile as tile
from concourse import bacc, mybir
from concourse import bass_utils

B, C, S, HW = 32, 1280, 16, 256
P = 128
KC = C // P          # 10 chunks of the channel dim
NCORES = 8
BL = B // NCORES     # 4 samples per core
SHIFT = 45.0
CGROUPS = [(0, 512), (512, 512), (1024, 256)]  # psum-bank-sized col groups
F32 = mybir.dt.float32
BF16 = mybir.dt.bfloat16
AF = mybir.ActivationFunctionType

_CACHE = {}
# AllReduce chunking: list of (start_dt, n_dt); chunk issued after its
# last dt's S row is stored.
AR_MODE = os.environ.get("KERNEL_AR_MODE", "split3")
SCHUNKS = {
    "split3": [(0, 3), (3, 3), (6, 4)],
    "split2": [(0, 5), (5, 5)],
    "single": [(0, 10)],
}[AR_MODE]


def _emit(nc, tc, io, alpha):
    ones, bvc, brc = io["ones_t"], io["bvc_t"], io["brc_t"]
    xb_d = io["xb_d"]
    wk_d, wq_d, wv_d, wr_d = io["wk_d"], io["wq_d"], io["wv_d"], io["wr_d"]
    out_d = io["out_d"]

    # ---------------- pools: left stack base ----------------
    xbp_ctx = tc.tile_pool(name="xbp", bufs=1, side="left")
    xbp = xbp_ctx.__enter__()
    xb = xbp.tile([P, KC, BL * HW], BF16, tag="xb")    # 20 KB/p, whole kernel
    ktqt_ctx = tc.tile_pool(name="ktqt", bufs=1, side="left")
    ktqtp = ktqt_ctx.__enter__()
    kt = ktqtp.tile([P, 2, BL, C], BF16, tag="kt")     # 20 KB/p
    qt = ktqtp.tile([P, 2, BL, C], BF16, tag="qt")     # 20 KB/p
    wk_ctx = tc.tile_pool(name="wkp", bufs=1, side="left")
    wkp = wk_ctx.__enter__()
    wk_sb = wkp.tile([P, KC, C], BF16, tag="wk")       # 25.6 KB/p
    brow_k = wkp.tile([1, C], BF16, tag="browk")

    # load order on SP: wk, bias row, then xb per-sample (b0 first)
    nc.sync.dma_start(wk_sb[:], wk_d.ap().rearrange("(k p) n -> p k n", p=P))
    nc.sync.dma_start(brow_k[:], io["bk"].ap())
    for b in range(BL):
        nc.sync.dma_start(
            xb[:, :, b * HW:(b + 1) * HW],
            xb_d.ap()[b].rearrange("(k p) n -> p k n", p=P),
        )

    def proj_group(dest, w_sb, brow, cgs, cgl, b, hwt, psp):
        """dest[:, hwt, b, cgs:cgs+cgl] = (x_b^T W)[hw-chunk, cg] + bias."""
        ps = psp.tile([P, 512], F32, tag="psA")
        for k in range(KC):
            nc.tensor.matmul(
                ps[:, :cgl],
                xb[:, k, b * HW + hwt * P:b * HW + (hwt + 1) * P],
                w_sb[:, k, cgs:cgs + cgl],
                start=(k == 0),
                stop=False,
            )
        nc.tensor.matmul(
            ps[:, :cgl], ones[:, :P], brow[:, cgs:cgs + cgl],
            start=False, stop=True,
        )
        nc.scalar.copy(dest[:, hwt, b, cgs:cgs + cgl], ps[:, :cgl])

    # ---------------- kt (whole) ----------------
    psA_ctx = tc.tile_pool(name="psA", bufs=2, space="PSUM", side="left")
    psA = psA_ctx.__enter__()
    for cgs, cgl in CGROUPS:
        for b in range(BL):
            for hwt in range(2):
                proj_group(kt, wk_sb, brow_k, cgs, cgl, b, hwt, psA)
    wk_ctx.__exit__(None, None, None)

    # ---------------- E + wq pools (right) ----------------
    ep_ctx = tc.tile_pool(name="ep", bufs=1, side="right")
    ep = ep_ctx.__enter__()
    e_sb = ep.tile([P, BL, KC, C], BF16, tag="E")      # 102.4 KB/p
    wq_ctx = tc.tile_pool(name="wqp", bufs=1, side="right")
    wqp = wq_ctx.__enter__()
    wq_sb = wqp.tile([P, KC, C], BF16, tag="wq")       # 25.6 KB/p
    brow_q = wqp.tile([1, C], BF16, tag="browq")
    nc.sync.dma_start(wq_sb[:], wq_d.ap().rearrange("(k p) n -> p k n", p=P))
    nc.sync.dma_start(brow_q[:], io["bq"].ap())

    # ---------------- qt cgroup0 ----------------
    cg0s, cg0l = CGROUPS[0]
    for b in range(BL):
        for hwt in range(2):
            proj_group(qt, wq_sb, brow_q, cg0s, cg0l, b, hwt, psA)

    # qt cgroups 1,2 remaining groups, interleaved into B's dt loop
    qt_tail = [
        (cgs, cgl, b, hwt)
        for cgs, cgl in CGROUPS[1:]
        for b in range(BL)
        for hwt in range(2)
    ]
    qt_per_dt = {0: 3, 1: 3, 2: 2, 3: 3, 4: 3, 5: 2}

    # which AR chunk owns dt, and the chunk-local row index
    dt2chunk = {}
    for ci, (d0, nd) in enumerate(SCHUNKS):
        for j in range(nd):
            dt2chunk[d0 + j] = (ci, j)

    # ---------------- fused B ----------------
    sst_ctx = tc.tile_pool(name="sst", bufs=1, side="left")
    sstp = sst_ctx.__enter__()                          # 7.5 KB/p
    psB_ctx = tc.tile_pool(name="psB", bufs=3, space="PSUM", side="right")
    psB = psB_ctx.__enter__()
    qi = 0
    for dt in range(KC):
        for b in range(BL):
            for cgs, cgl in CGROUPS:
                ps = psB.tile([P, 512], F32, tag="psB")
                for hwt in range(2):
                    nc.tensor.matmul(
                        ps[:, :cgl],
                        qt[:, hwt, b, dt * P:(dt + 1) * P],
                        kt[:, hwt, b, cgs:cgs + cgl],
                        start=(hwt == 0),
                        stop=(hwt == 1),
                    )
                nc.scalar.activation(
                    e_sb[:, b, dt, cgs:cgs + cgl], ps[:, :cgl],
                    AF.Exp, bias=-SHIFT, scale=1.0,
                )
        # S[dt] = (E0+E1) + (E2+E3), bf16 pairwise tree on DVE
        s01 = sstp.tile([P, C], BF16, tag="s01")
        s23 = sstp.tile([P, C], BF16, tag="s23")
        st = sstp.tile([P, C], BF16, tag="st")
        nc.vector.tensor_add(s01[:], e_sb[:, 0, dt], e_sb[:, 1, dt])
        nc.vector.tensor_add(s23[:], e_sb[:, 2, dt], e_sb[:, 3, dt])
        nc.vector.tensor_add(st[:], s01[:], s23[:])
        ci, j = dt2chunk[dt]
        nc.sync.dma_start(io["s_in"][ci].ap()[j], st[:])
        # interleave remaining qt groups to keep PE ahead of ACT exp
        for _ in range(qt_per_dt.get(dt, 0)):
            cgs, cgl, b, hwt = qt_tail[qi]
            qi += 1
            proj_group(qt, wq_sb, brow_q, cgs, cgl, b, hwt, psA)
        if dt == 5:
            psA_ctx.__exit__(None, None, None)
            wq_ctx.__exit__(None, None, None)
        # AR chunk issues (gpsimd queue holds only the 3 collectives)
        for cix, (d0, nd) in enumerate(SCHUNKS):
            if dt == d0 + nd - 1:
                nc.gpsimd.collective_compute(
                    "AllReduce",
                    mybir.AluOpType.add,
                    replica_groups=[list(range(NCORES))],
                    ins=[io["s_in"][cix].ap()],
                    outs=[io["s_out"][cix].ap()],
                )
    assert qi == len(qt_tail)
    psB_ctx.__exit__(None, None, None)
    sst_ctx.__exit__(None, None, None)
    ktqt_ctx.__exit__(None, None, None)

    # ---------------- C pools ----------------
    # right: V, z, r above E;  left: wr, o, att, wv above xb
    vp_ctx = tc.tile_pool(name="vp", bufs=1, side="right")
    vp = vp_ctx.__enter__()
    v_sb = vp.tile([P, KC, BL * HW], BF16, tag="V")     # 20 KB/p
    zb_ctx = tc.tile_pool(name="zbp", bufs=1, side="right")
    zbp = zb_ctx.__enter__()                            # 2.5 KB/p
    rp_ctx = tc.tile_pool(name="rp", bufs=2, side="right")
    rp = rp_ctx.__enter__()                             # 5 KB/p
    zf_ctx = tc.tile_pool(name="zfp", bufs=2, side="right")
    zfp = zf_ctx.__enter__()                            # 4 KB/p
    wr_ctx = tc.tile_pool(name="wrp", bufs=1, side="left")
    wrp = wr_ctx.__enter__()
    wr_sb = wrp.tile([P, KC, C], BF16, tag="wr")        # 25.6 KB/p
    o_ctx = tc.tile_pool(name="op", bufs=3, side="left")
    op = o_ctx.__enter__()                              # 3 KB/p
    att_ctx = tc.tile_pool(name="attp", bufs=1, side="left")
    attp = att_ctx.__enter__()
    att_sb = attp.tile([P, KC, BL * HW], BF16, tag="att")  # 20 KB/p
    wv_ctx = tc.tile_pool(name="wvp", bufs=2, side="left")
    wvp = wv_ctx.__enter__()                            # 5 KB/p
    psV_ctx = tc.tile_pool(name="psV", bufs=2, space="PSUM", side="left")
    psV = psV_ctx.__enter__()
    psR_ctx = tc.tile_pool(name="psR", bufs=1, space="PSUM", side="right")
    psR = psR_ctx.__enter__()

    # wv chunk loads (per vct) + wr load early on SP
    def wv_load(vct):
        t = wvp.tile([P, KC, P], BF16, tag="wvc", name=f"wv{vct}")
        nc.sync.dma_start(
            t[:],
            wv_d.ap()[:, vct * P:(vct + 1) * P].rearrange(
                "(k p) n -> p k n", p=P),
        )
        return t

    wv0 = wv_load(0)
    nc.sync.dma_start(wr_sb[:], wr_d.ap().rearrange("(k p) n -> p k n", p=P))

    def recip_and_muls(dt):
        """z[dt] -> f32 -> 1/z -> E[:, :, dt, :] *= R  (in place)."""
        ci, j = dt2chunk[dt]
        zb = zbp.tile([P, C], BF16, tag="zb")
        nc.sync.dma_start(zb[:], io["s_out"][ci].ap()[j])
        r = rp.tile([P, C], BF16, tag="r")
        for cgs, cgl in CGROUPS:
            # NR step may read only one non-scalar input from PSUM:
            # z lives in SBUF f32, scratch + result in PSUM.
            zf = zfp.tile([P, 512], F32, tag="zf")
            scr = psR.tile([P, 512], F32, tag="scr")
            rf = psR.tile([P, 512], F32, tag="rf")
            nc.scalar.copy(zf[:, :cgl], zb[:, cgs:cgs + cgl])
            nc.vector.reciprocal_approx_accurate(
                rf[:, :cgl], zf[:, :cgl], scr[:, :cgl]
            )
            nc.scalar.copy(r[:, cgs:cgs + cgl], rf[:, :cgl])
        for b in range(BL):
            eng = nc.gpsimd if b % 2 == 1 else nc.vector
            eng.tensor_mul(e_sb[:, b, dt], e_sb[:, b, dt], r[:])

    def v_gemms(vct, wvt):
        for bp in range(2):
            ps = psV.tile([P, 512], F32, tag="psV")
            for ci_ in range(KC):
                nc.tensor.matmul(
                    ps[:],
                    wvt[:, ci_, :],
                    xb[:, ci_, bp * 512:(bp + 1) * 512],
                    start=(ci_ == 0),
                    stop=(ci_ == KC - 1),
                )
            nc.vector.tensor_scalar_add(
                v_sb[:, vct, bp * 512:(bp + 1) * 512], ps[:],
                bvc[:, vct:vct + 1],
            )

    # V GEMMs with recip/mul chains pipelined behind the AR landings
    wv_next = None
    for vct in range(KC):
        wvt = wv0 if vct == 0 else wv_next
        if vct + 1 < KC:
            wv_next = wv_load(vct + 1)
        v_gemms(vct, wvt)
        if vct <= 5:
            recip_and_muls(vct)
    for dt in range(6, KC):
        recip_and_muls(dt)
    wv_ctx.__exit__(None, None, None)
    psV_ctx.__exit__(None, None, None)

    # ---------------- C2/C3: att + refine, interleaved ----------------
    attps_ctx = tc.tile_pool(name="attps", bufs=3, space="PSUM", side="left")
    attps = attps_ctx.__enter__()

    def att_gemms(b):
        for ct in range(KC):
            ps = attps.tile([P, HW], F32, tag="psAtt")
            for dt in range(KC):
                nc.tensor.matmul(
                    ps[:],
                    e_sb[:, b, dt, ct * P:(ct + 1) * P],
                    v_sb[:, dt, b * HW:(b + 1) * HW],
                    start=(dt == 0),
                    stop=(dt == KC - 1),
                )
            nc.scalar.copy(att_sb[:, ct, b * HW:(b + 1) * HW], ps[:])

    out_ap = out_d.ap().rearrange("b (k p) n -> p k b n", p=P)

    def refine(bp, refps):
        for ot in range(KC):
            ps = refps.tile([P, 512], F32, tag="psRef")
            for ct in range(KC):
                nc.tensor.matmul(
                    ps[:],
                    wr_sb[:, ct, ot * P:(ot + 1) * P],
                    att_sb[:, ct, bp * 512:(bp + 1) * 512],
                    start=(ct == 0),
                    stop=(ct == KC - 1),
                )
            for j in range(2):
                b = 2 * bp + j
                o = op.tile([P, HW], F32, tag="o")
                # out = alpha * psum + (alpha*br + x)
                nc.vector.affine_then_add(
                    o[:], ps[:, j * HW:(j + 1) * HW],
                    xb[:, ot, b * HW:(b + 1) * HW],
                    scale=alpha, bias=brc[:, ot:ot + 1],
                )
                nc.sync.dma_start(out_ap[:, ot, b, :], o[:])

    att_gemms(0)
    att_gemms(1)
    psR_ctx.__exit__(None, None, None)
    refps_ctx = tc.tile_pool(name="refps", bufs=2, space="PSUM", side="right")
    refps = refps_ctx.__enter__()
    refine(0, refps)
    att_gemms(2)
    att_gemms(3)
    refine(1, refps)

    refps_ctx.__exit__(None, None, None)
    attps_ctx.__exit__(None, None, None)
    att_ctx.__exit__(None, None, None)
    o_ctx.__exit__(None, None, None)
    wr_ctx.__exit__(None, None, None)
    zf_ctx.__exit__(None, None, None)
    rp_ctx.__exit__(None, None, None)
    zb_ctx.__exit__(None, None, None)
    vp_ctx.__exit__(None, None, None)
    ep_ctx.__exit__(None, None, None)
    xbp_ctx.__exit__(None, None, None)


def build(alpha: float, nrep: int = 1):
    nc = bacc.Bacc(
        "TRN2",
        target_bir_lowering=False,
        debug=False,
        enable_asserts=False,
        num_devices=NCORES,
    )

    io = {}
    io["xb_d"] = nc.dram_tensor("xb", [BL, C, HW], BF16, kind="ExternalInput")
    io["wk_d"] = nc.dram_tensor("wkt", [C, C], BF16, kind="ExternalInput")  # Wk.T
    io["wq_d"] = nc.dram_tensor("wqt", [C, C], BF16, kind="ExternalInput")
    io["wv_d"] = nc.dram_tensor("wvt", [C, C], BF16, kind="ExternalInput")
    io["wr_d"] = nc.dram_tensor("wrt", [C, C], BF16, kind="ExternalInput")
    for nm in ("bk", "bq"):
        io[nm] = nc.dram_tensor(nm, [1, C], BF16, kind="ExternalInput")
    io["bvc"] = nc.dram_tensor("bvc", [P, KC], F32, kind="ExternalInput")
    io["brc"] = nc.dram_tensor("brc", [P, KC], F32, kind="ExternalInput")
    io["ones_d"] = nc.dram_tensor("ones", [1, HW], BF16, kind="ExternalInput")
    io["out_d"] = nc.dram_tensor("out", [BL, C, HW], F32, kind="ExternalOutput")

    io["s_in"] = [
        nc.dram_tensor(f"s_in{i}", [nd, P, C], BF16)
        for i, (_, nd) in enumerate(SCHUNKS)
    ]
    io["s_out"] = [
        nc.dram_tensor(f"s_out{i}", [nd, P, C], BF16, addr_space="Shared")
        for i, (_, nd) in enumerate(SCHUNKS)
    ]

    # const AP so ACT Exp can take bias=-SHIFT
    cshift = nc.alloc_sbuf_tensor("const-shift", [128, 1], F32)
    nc.gpsimd.memset(cshift.ap(), -SHIFT)
    nc.const_aps.aps[(F32, -SHIFT)] = cshift.ap()
    nc.all_engine_barrier()

    with tile.TileContext(nc) as tc:
        with tc.tile_pool(name="cpool", bufs=1, side="left") as cpool:
            ones = cpool.tile([1, HW], BF16, tag="ones")
            nc.sync.dma_start(ones[:], io["ones_d"].ap())
            bvc = cpool.tile([P, KC], F32, tag="bvc")
            nc.sync.dma_start(bvc[:], io["bvc"].ap())
            brc = cpool.tile([P, KC], F32, tag="brc")
            nc.sync.dma_start(brc[:], io["brc"].ap())
            io["ones_t"] = ones
            io["bvc_t"] = bvc
            io["brc_t"] = brc

            for _ in range(nrep):
                _emit(nc, tc, io, alpha)

    nc.compile()
    return nc


def make_in_maps(x, Wq, bq, Wk, bk, Wv, bv, Wr, br, alpha=0.1):
    bf = ml_dtypes.bfloat16
    alpha_f = float(np.asarray(alpha).reshape(-1)[0])
    xsb = np.asarray(x, dtype=np.float32).reshape(B, C, HW).astype(bf)
    w = {
        "wkt": np.ascontiguousarray(np.asarray(Wk, dtype=np.float32).T.astype(bf)),
        "wqt": np.ascontiguousarray(np.asarray(Wq, dtype=np.float32).T.astype(bf)),
        "wvt": np.ascontiguousarray(np.asarray(Wv, dtype=np.float32).T.astype(bf)),
        "wrt": np.ascontiguousarray(np.asarray(Wr, dtype=np.float32).T.astype(bf)),
    }
    rows = {
        "bk": np.asarray(bk, dtype=np.float32).reshape(1, C).astype(bf),
        "bq": np.asarray(bq, dtype=np.float32).reshape(1, C).astype(bf),
    }
    # per-partition bias columns: [P, KC]; chunk ct holds channels
    # ct*P..(ct+1)*P-1 in partition order
    bvc = np.ascontiguousarray(
        np.asarray(bv, dtype=np.float32).reshape(KC, P).T)
    brc = np.ascontiguousarray(
        (alpha_f * np.asarray(br, dtype=np.float32)).reshape(KC, P).T)
    in_maps = []
    for c in range(NCORES):
        in_maps.append({
            "xb": np.ascontiguousarray(xsb[c * BL:(c + 1) * BL]),
            **w,
            "ones": np.ones((1, HW), dtype=bf),
            "bvc": bvc,
            "brc": brc,
            **rows,
        })
    return in_maps


def kernel(x, Wq, bq, Wk, bk, Wv, bv, Wr, br, alpha):
    alpha_f = float(np.asarray(alpha).reshape(-1)[0])
    key = ("v5", alpha_f, AR_MODE)
    if key not in _CACHE:
        _CACHE[key] = build(alpha_f)
    nc = _CACHE[key]

    in_maps = make_in_maps(x, Wq, bq, Wk, bk, Wv, bv, Wr, br, alpha_f)
    res = bass_utils.run_bass_kernel_spmd(nc, in_maps, core_ids=list(range(NCORES)))
    out = np.concatenate([res.results[c]["out"] for c in range(NCORES)], axis=0)
    return np.ascontiguousarray(out.reshape(B, C, S, S).astype(np.float32))
